# revision 12
# baseline (speedup 1.0000x reference)
"""Trainium2 Bass kernel for AMPBlock2 (dense_cnn): 8-way batch-parallel.

Each of the 8 NeuronCores processes one batch sample [512, 8192] end-to-end.

v3: transposed-layout banded-matmul filters + 2a-premultiplied input.

Key algebra: feed the pipeline x2a = 2a*x instead of x. Then
  y2   = Up(x2a) = 2a*Up(x)      -> sin arg needs no per-channel scale
  cos  = sin(y2 - 2pi*round(y2/2pi) + pi/2)
  z'   = G*x2a - Down(cos)       since 2a*c1 = a/(a+1e-9) ~= 1 (exact to 1e-9)
  out  = W' (*)_d z' + b_eff + x with W'[o,i,k] = W[o,i,k]/(2a[i]) host-folded
so the per-channel snake scalings cost ZERO device ops. x2a for block 1 is
precomputed on host; block 2's x2a is produced by an ACT copy-with-scale
(per-partition 2a in C-layout) and staged through DRAM in bf16.

Per 128-time tile (stride 104/96, halo 6+6), all moving ops [128, 512+]:
  PE : 4 bf16 transposes (x2a^T), 2 up matmuls -> y-pair [128,1024] PSUM,
       3 matmuls (G + down_e + down_o) -> z'^T, 4 bf16 back-transposes
  DVE: k = round(y2/2pi + .25) (i32), w2 = y2 - 2pi*k   (phase-paired)
  ACT: x2a^T evac, sin(w2 + pi/2) -> c (paired), z' evac
Per 512-col segment: conv 48 bf16 matmuls + residual STT + DMA.

Edge columns (first/last 32) are recomputed exactly on host.
"""
import os, sys
sys.path.insert(0, '/opt/trn_rl_repo')
import numpy as np
from contextlib import ExitStack

import concourse.bass as bass
import concourse.bacc as bacc
import concourse.tile as tile
from concourse import mybir
from concourse.bass_utils import run_bass_kernel_spmd
import ml_dtypes

F32 = mybir.dt.float32
F32R = mybir.dt.float32r
BF16 = mybir.dt.bfloat16
I32 = mybir.dt.int32
AF = mybir.ActivationFunctionType
ALU = mybir.AluOpType

C, T = 512, 8192
NCH = 4              # channel chunks of 128
SEG = 512            # conv segment width
NSEG = T // SEG      # 16
SUB = [(0, 104), (104, 104), (208, 104), (312, 104), (416, 96)]  # (off, valid)
XW = SEG + 32        # x2a seg width: [seg*512-6, +544)
ZW = SEG + 6         # zseg per-chunk width: [seg*512-3, +515)
PAD = 32             # edge-replicated padding cols in DRAM x2a arrays
TWO_PI = float(2 * np.pi)
INV_2PI = float(1.0 / (2 * np.pi))
HALF_PI = float(np.pi / 2)

# ---------------- host-side filter derivation (pure numpy) ----------------

def _kaiser_sinc_filter1d(cutoff, half_width, kernel_size):
    half_size = kernel_size // 2
    delta_f = 4.0 * half_width
    A = 2.285 * (half_size - 1) * np.pi * delta_f + 7.95
    if A > 50.0:
        beta = 0.1102 * (A - 8.7)
    elif A >= 21.0:
        beta = 0.5842 * (A - 21.0) ** 0.4 + 0.07886 * (A - 21.0)
    else:
        beta = 0.0
    window = np.kaiser(kernel_size, beta)
    time = np.arange(-half_size, half_size) + 0.5
    filt = 2.0 * cutoff * window * np.sinc(2.0 * cutoff * time)
    return (filt / filt.sum()).astype(np.float64)

FILT = _kaiser_sinc_filter1d(0.25, 0.3, 12)
FE_U = 2.0 * FILT[[0, 2, 4, 6, 8, 10]]   # y_e[t] = sum_j FE_U[j] x[t-3+j]
FO_U = 2.0 * FILT[[1, 3, 5, 7, 9, 11]]   # y_o[t] = sum_j FO_U[j] x[t-2+j]
FE_D = FILT[[1, 3, 5, 7, 9, 11]]         # z[t] += sum_m FE_D[m] s_e[t+m-2]
FO_D = FILT[[0, 2, 4, 6, 8, 10]]         # z[t] += sum_m FO_D[m] s_o[t+m-3]
G = np.zeros(11)                          # z_lin[t] = sum_r G[r] x[t+r-5]
for _m in range(6):
    for _j in range(6):
        G[_m + _j] += FE_D[_m] * FE_U[_j] + FO_D[_m] * FO_U[_j]

# ---------------- device kernel build (shape-static, cached) ----------------

_NC_CACHE = {}


def _build():
    nc = bacc.Bacc("TRN2", target_bir_lowering=False, debug=False)

    x2a1 = nc.declare_dram_parameter("x2a1", [C, T + 2 * PAD], BF16, isOutput=False)
    sup = nc.declare_dram_parameter("sup", [2, 128, 128], BF16, isOutput=False)
    sdn = nc.declare_dram_parameter("sdn", [2, 128, 128], BF16, isOutput=False)
    sg = nc.declare_dram_parameter("sg", [128, 128], BF16, isOutput=False)
    idb = nc.declare_dram_parameter("idb", [128, 128], BF16, isOutput=False)
    a2sc = nc.declare_dram_parameter("a2sc", [NCH, 128, 1], F32, isOutput=False)
    inv2asc = nc.declare_dram_parameter("inv2asc", [2, NCH, 128, 1], F32, isOutput=False)
    beff = nc.declare_dram_parameter("beff", [2, NCH, 128, 1], F32, isOutput=False)
    wconv = nc.declare_dram_parameter("wconv", [2, 3, NCH, 128, C], BF16, isOutput=False)
    out = nc.declare_dram_parameter("out", [C, T], F32, isOutput=True)

    with ExitStack() as ctx:
        tc = ctx.enter_context(tile.TileContext(nc))

        cpool = ctx.enter_context(tc.tile_pool(name="consts", bufs=1))
        dram = ctx.enter_context(tc.tile_pool(name="dram", bufs=1, space="DRAM"))
        xpool = ctx.enter_context(tc.tile_pool(name="x", bufs=3))
        x2spool = ctx.enter_context(tc.tile_pool(name="x2s", bufs=3))
        xtpool = ctx.enter_context(tc.tile_pool(name="xt", bufs=3))
        kpool = ctx.enter_context(tc.tile_pool(name="k", bufs=2))
        wpool = ctx.enter_context(tc.tile_pool(name="w", bufs=2))
        cpool2 = ctx.enter_context(tc.tile_pool(name="c", bufs=3))
        zbpool = ctx.enter_context(tc.tile_pool(name="zb", bufs=3))
        zpool = ctx.enter_context(tc.tile_pool(name="z", bufs=3))
        xnpool = ctx.enter_context(tc.tile_pool(name="xn", bufs=3))
        x2npool = ctx.enter_context(tc.tile_pool(name="x2n", bufs=3))
        pypool = ctx.enter_context(tc.tile_pool(name="py", bufs=2, space="PSUM"))
        pzpool = ctx.enter_context(tc.tile_pool(name="pzT", bufs=1, space="PSUM"))
        pzbpool = ctx.enter_context(tc.tile_pool(name="pzb", bufs=1, space="PSUM"))
        popool = ctx.enter_context(tc.tile_pool(name="po", bufs=2, space="PSUM"))

        # ---- load constants into SBUF ----
        sup_t = [cpool.tile([128, 128], BF16, name=f"sup{p}", tag=f"sup{p}") for p in range(2)]
        sdn_t = [cpool.tile([128, 128], BF16, name=f"sdn{p}", tag=f"sdn{p}") for p in range(2)]
        for p in range(2):
            nc.sync.dma_start(sup_t[p][:], sup[p])
            nc.sync.dma_start(sdn_t[p][:], sdn[p])
        sg_t = cpool.tile([128, 128], BF16, name="sg", tag="sg")
        nc.sync.dma_start(sg_t[:], sg[:, :])
        idb_t = cpool.tile([128, 128], BF16, name="idb", tag="idb")
        nc.sync.dma_start(idb_t[:], idb[:, :])
        a2sc_t = [cpool.tile([128, 1], F32, name=f"a2s{co}", tag=f"a2s{co}") for co in range(NCH)]
        for co in range(NCH):
            nc.sync.dma_start(a2sc_t[co][:], a2sc[co])
        inv2a_t = [[cpool.tile([128, 1], F32, name=f"i2a{b}{co}", tag=f"i2a{b}{co}")
                    for co in range(NCH)] for b in range(2)]
        for b in range(2):
            for co in range(NCH):
                nc.sync.dma_start(inv2a_t[b][co][:], inv2asc[b, co])
        hpi_t = cpool.tile([128, 1], F32, name="hpi", tag="hpi")
        nc.vector.memset(hpi_t[:], HALF_PI)
        beff_t = [[cpool.tile([128, 1], F32, name=f"be{b}{co}", tag=f"be{b}{co}")
                   for co in range(NCH)] for b in range(2)]
        for b in range(2):
            for co in range(NCH):
                nc.sync.dma_start(beff_t[b][co][:], beff[b, co])
        w_t = [[[cpool.tile([128, C], BF16, name=f"w{b}{k}{ci}", tag=f"w{b}{k}{ci}")
                 for ci in range(NCH)] for k in range(3)] for b in range(2)]
        for b in range(2):
            for k in range(3):
                for ci in range(NCH):
                    nc.sync.dma_start(w_t[b][k][ci][:], wconv[b, k, ci])

        x2stage = dram.tile([C, T + 2 * PAD], BF16, name="x2stage")

        def emit_tiles(b, seg, x2src, state):
            t_lo = seg * SEG - 6
            x2seg = []
            for ci in range(NCH):
                x2_t = x2spool.tile([128, XW], BF16, name=f"x2s{ci}", tag=f"x2s{ci}")
                nc.sync.dma_start(x2_t[:],
                                  x2src[128 * ci:128 * (ci + 1),
                                        PAD + t_lo:PAD + t_lo + XW])
                x2seg.append(x2_t)
            state["x2segs"][seg] = x2seg

            zseg = zpool.tile([128, NCH * ZW], BF16, name="zs", tag="zs")
            z3 = zseg[:].rearrange("p (c t) -> p c t", c=NCH)
            if seg == 0:
                nc.vector.memset(z3[:, :, 0:3], 0.0)
            if seg == NSEG - 1:
                nc.vector.memset(z3[:, :, ZW - 3:ZW], 0.0)
            if seg > 0:
                # left halo: t in [seg*512-3, seg*512) lives at prev cols [512, 515)
                zprev3 = state["zseg"][:].rearrange("p (c t) -> p c t", c=NCH)
                nc.vector.tensor_copy(z3[:, :, 0:3], zprev3[:, :, SEG:SEG + 3])

            for (off, V) in SUB:
                xt = xtpool.tile([128, 512], BF16, name="xt", tag="xt")
                nc.sync.dma_start(xt[:],
                                  x2src[0:C, PAD + t_lo + off:PAD + t_lo + off + 128],
                                  transpose=True)

                py_ = pypool.tile([128, 1024], F32, name="py", tag="py")
                nc.tensor.matmul(py_[:, 0:512], sup_t[0][:], xt[:], start=True, stop=True)
                nc.tensor.matmul(py_[:, 512:1024], sup_t[1][:], xt[:], start=True, stop=True)
                k_ = kpool.tile([128, 1024], I32, name="k", tag="k")
                nc.vector.tensor_scalar(k_[:], py_[:], INV_2PI, 0.25, ALU.mult, ALU.add)
                w_ = wpool.tile([128, 1024], F32, name="w", tag="w")
                nc.vector.scalar_tensor_tensor(w_[:], k_[:], -TWO_PI, py_[:],
                                               ALU.mult, ALU.add)
                c_ = cpool2.tile([128, 1024], BF16, name="c", tag="c")
                nc.scalar.activation(c_[:], w_[:], AF.Sin, bias=hpi_t[:, 0:1], scale=1.0)

                pzT = pzpool.tile([128, 512], F32, name="pzT", tag="pzT")
                nc.tensor.matmul(pzT[:], sg_t[:], xt[:], start=True, stop=False)
                nc.tensor.matmul(pzT[:], sdn_t[0][:], c_[:, 0:512], start=False, stop=False)
                nc.tensor.matmul(pzT[:], sdn_t[1][:], c_[:, 512:1024], start=False, stop=True)
                zbf = zbpool.tile([128, 512], BF16, name="zbf", tag="zbf")
                nc.scalar.copy(zbf[:], pzT[:])
                pzb = pzbpool.tile([128, 512], BF16, name="pzb", tag="pzb")
                for ci in range(NCH):
                    nc.tensor.transpose(pzb[:, 128 * ci:128 * (ci + 1)],
                                        zbf[:, 128 * ci:128 * (ci + 1)], idb_t[:])
                pz3 = pzb[:].rearrange("p (c t) -> p c t", c=NCH)
                # valid z rows [6, 6+V) -> t in [seg*512+off, +V) -> zseg col off+3
                nc.scalar.copy(z3[:, :, off + 3:off + 3 + V], pz3[:, :, 6:6 + V])
                if off == 0 and seg > 0:
                    # right halo of previous zseg: t in [seg*512, +3)
                    nc.vector.tensor_copy(zprev3[:, :, SEG + 3:SEG + 6], pz3[:, :, 6:9])

            state["zseg"] = zseg

        def emit_conv(b, seg, dil, dst, state):
            zseg = state["zsegs"][seg]
            z3 = zseg[:].rearrange("p (c t) -> p c t", c=NCH)
            x2seg = state["x2segs"].pop(seg)
            t0 = seg * SEG
            for co in range(NCH):
                # reconstruct the f32 residual from the staged 2a*x bf16 tile
                xr = xpool.tile([128, 512], F32, name=f"xr{co}", tag=f"xr{co}")
                nc.scalar.activation(xr[:], x2seg[co][:, 6:6 + SEG], AF.Copy,
                                     bias=0.0, scale=inv2a_t[b][co][:, 0:1])
                po = popool.tile([128, 512], F32, name="po", tag="po")
                n = 0
                for k in range(3):
                    for ci in range(NCH):
                        zofs = 3 + (k - 1) * dil
                        nc.tensor.matmul(po[:],
                                         w_t[b][k][ci][:, 128 * co:128 * (co + 1)],
                                         z3[:, ci, zofs:zofs + 512],
                                         start=(n == 0), stop=(n == 11))
                        n += 1
                xn = xnpool.tile([128, 512], F32, name="xn", tag="xn")
                nc.vector.scalar_tensor_tensor(xn[:], po[:], beff_t[b][co][:, 0:1],
                                               xr[:],
                                               ALU.add, ALU.add)
                if b == 0:
                    # stage 2*a2*x' in bf16 for block 2's filter + residual path
                    x2n = x2npool.tile([128, 512], BF16, name="x2n", tag="x2n")
                    nc.scalar.activation(x2n[:], xn[:], AF.Copy, bias=0.0,
                                         scale=a2sc_t[co][:, 0:1])
                    nc.sync.dma_start(x2stage[128 * co:128 * (co + 1),
                                              PAD + t0:PAD + t0 + 512], x2n[:])
                    if seg == 0:
                        ep = x2npool.tile([128, PAD], BF16, name="epl", tag="epl")
                        nc.vector.tensor_copy(ep[:], x2n[:, 0:1].to_broadcast([128, PAD]))
                        nc.sync.dma_start(x2stage[128 * co:128 * (co + 1), 0:PAD], ep[:])
                    if seg == NSEG - 1:
                        ep = x2npool.tile([128, PAD], BF16, name="epr", tag="epr")
                        nc.vector.tensor_copy(ep[:], x2n[:, 511:512].to_broadcast([128, PAD]))
                        nc.sync.dma_start(x2stage[128 * co:128 * (co + 1),
                                                  PAD + T:PAD + T + PAD], ep[:])
                else:
                    nc.sync.dma_start(dst[128 * co:128 * (co + 1), t0:t0 + 512], xn[:])

        for b in range(2):
            dil = 1 if b == 0 else 3
            x2src = x2a1 if b == 0 else x2stage
            state = {"zseg": None, "zsegs": {}, "x2segs": {}}
            for seg in range(NSEG + 1):
                if seg < NSEG:
                    emit_tiles(b, seg, x2src, state)
                    state["zsegs"][seg] = state["zseg"]
                if seg >= 1:
                    emit_conv(b, seg - 1, dil, out, state)
                    state["zsegs"].pop(seg - 1)

    nc.finalize()
    return nc


def _get_nc():
    if "nc" not in _NC_CACHE:
        _NC_CACHE["nc"] = _build()
    return _NC_CACHE["nc"]


# ---------------- host prep ----------------

def _prep_consts(w1, b1, alpha1, w2, b2, alpha2):
    sup = np.zeros((2, 128, 128), np.float32)
    for m in range(128):
        for j in range(6):
            k = m - 3 + j
            if 0 <= k < 128:
                sup[0, k, m] = FE_U[j]
            k = m - 2 + j
            if 0 <= k < 128:
                sup[1, k, m] = FO_U[j]
    sdn = np.zeros((2, 128, 128), np.float32)
    for m in range(128):
        for j in range(6):
            k = m + j - 2
            if 0 <= k < 128:
                sdn[0, k, m] = -FE_D[j]
            k = m + j - 3
            if 0 <= k < 128:
                sdn[1, k, m] = -FO_D[j]
    sg = np.zeros((128, 128), np.float32)
    for m in range(128):
        for r in range(11):
            k = m - 5 + r
            if 0 <= k < 128:
                sg[k, m] = G[r]

    a2sc = np.zeros((NCH, 128, 1), np.float32)
    beff = np.zeros((2, NCH, 128, 1), np.float32)
    wconv = np.zeros((2, 3, NCH, 128, C), np.float32)
    a2_all = []
    for b, (w, bb, alpha) in enumerate(((w1, b1, alpha1), (w2, b2, alpha2))):
        a = np.exp(alpha.astype(np.float64))
        a2 = 2.0 * a
        a2_all.append(a2)
        c1 = 0.5 / (a + 1e-9)
        b_eff = bb.astype(np.float64) + w.astype(np.float64).sum(axis=2) @ c1
        for co in range(NCH):
            beff[b, co, :, 0] = b_eff[128 * co:128 * (co + 1)].astype(np.float32)
        # fold 1/(2a_i) into the input-channel axis of W (z' = 2a*z on device)
        wdiv = w.astype(np.float64) / a2[None, :, None]
        for k in range(3):
            for ci in range(NCH):
                wconv[b, k, ci] = wdiv[:, 128 * ci:128 * (ci + 1), k].T.astype(np.float32)
    a2sc[:, :, 0] = a2_all[1].reshape(NCH, 128).astype(np.float32)
    inv2asc = np.zeros((2, NCH, 128, 1), np.float32)
    for b in range(2):
        inv2asc[b, :, :, 0] = (1.0 / a2_all[b]).reshape(NCH, 128).astype(np.float32)
    return dict(
        inv2asc=inv2asc,
        sup=sup.astype(ml_dtypes.bfloat16),
        sdn=sdn.astype(ml_dtypes.bfloat16),
        sg=sg.astype(ml_dtypes.bfloat16),
        idb=np.eye(128, dtype=np.float32).astype(ml_dtypes.bfloat16),
        a2sc=a2sc,
        beff=beff,
        wconv=wconv.astype(ml_dtypes.bfloat16),
    ), a2_all[0]


# ---------------- exact host reference for edge fix (pure numpy) ----------------

def _edge_pad(x, left, right):
    return np.concatenate([np.repeat(x[..., :1], left, -1), x,
                           np.repeat(x[..., -1:], right, -1)], -1)


def _ref_block_np(x, w, b, alpha, d):
    """Exact reference for one sub-block, x [B, C, S] float64. Valid in the
    interior and at TRUE array edges; inner cut edges are wrong (margin)."""
    B, Cc, S = x.shape
    a = np.exp(alpha)[None, :, None]
    xp = _edge_pad(x, 3, 3)
    y = np.zeros((B, Cc, 2 * S))
    for j in range(6):
        y[:, :, 0::2] += FE_U[j] * xp[:, :, j:j + S]
        y[:, :, 1::2] += FO_U[j] * xp[:, :, 1 + j:1 + j + S]
    s = y + np.sin(a * y) ** 2 / (a + 1e-9)
    sp = _edge_pad(s, 5, 6)
    z = np.zeros((B, Cc, S))
    for k in range(12):
        z += FILT[k] * sp[:, :, k:k + 2 * S:2][:, :, :S]
    zp = np.concatenate([np.zeros((B, Cc, d)), z, np.zeros((B, Cc, d))], -1)
    out = np.zeros((B, Cc, S))
    for k in range(3):
        out += np.einsum('oi,bit->bot', w[:, :, k], zp[:, :, k * d:k * d + S])
    return out + b[None, :, None] + x


def _ref_np(x, w1, b1, alpha1, w2, b2, alpha2):
    x = _ref_block_np(x, w1, b1, alpha1, 1)
    x = _ref_block_np(x, w2, b2, alpha2, 3)
    return x


EDGE_CTX = 96   # slice width used for edge recompute
EDGE_FIX = 32   # columns replaced at each end


def _fix_edges(out, x, w1, b1, alpha1, w2, b2, alpha2):
    args = (w1.astype(np.float64), b1.astype(np.float64), alpha1.astype(np.float64),
            w2.astype(np.float64), b2.astype(np.float64), alpha2.astype(np.float64))
    left = _ref_np(x[:, :, :EDGE_CTX].astype(np.float64), *args)
    right = _ref_np(x[:, :, -EDGE_CTX:].astype(np.float64), *args)
    out[:, :, :EDGE_FIX] = left[:, :, :EDGE_FIX].astype(np.float32)
    out[:, :, -EDGE_FIX:] = right[:, :, -EDGE_FIX:].astype(np.float32)
    return out


# ---------------- entry point ----------------

def run(inputs, trace=False):
    x = inputs["x"]
    consts, a2_1 = _prep_consts(inputs["w1"], inputs["b1"], inputs["alpha1"],
                                inputs["w2"], inputs["b2"], inputs["alpha2"])
    nc = _get_nc()
    in_maps = []
    for i in range(8):
        xi = np.ascontiguousarray(x[i])
        x2a1 = (a2_1[:, None].astype(np.float32) * xi).astype(ml_dtypes.bfloat16)
        x2a1 = np.pad(x2a1, ((0, 0), (PAD, PAD)), mode='edge')
        in_maps.append(dict(x2a1=np.ascontiguousarray(x2a1), **consts))
    res = run_bass_kernel_spmd(nc, in_maps, list(range(8)), trace=trace)
    out = np.stack([res.results[i]["out"] for i in range(8)]).astype(np.float32)
    out = _fix_edges(out, x, inputs["w1"], inputs["b1"], inputs["alpha1"],
                     inputs["w2"], inputs["b2"], inputs["alpha2"])
    return out, res


def kernel(x, w1, b1, alpha1, w2, b2, alpha2):
    out, _ = run(dict(x=x, w1=w1, b1=b1, alpha1=alpha1,
                      w2=w2, b2=b2, alpha2=alpha2))
    return out
